# revision 7
# baseline (speedup 1.0000x reference)
"""Trainium2 Bass kernel for nn_ContextEncoder (transformer encoder + per-agent mean).

Contract: kernel(**inputs) takes FULL inputs (as in reference.setup_inputs) and
returns the FULL [512, 256] agent_context output. Internally shards tokens
across 8 NeuronCores (sequence parallel), computes K/V replicated from an
AllGather of the residual stream, and does the segment mean via one-hot
matmuls with a host-side cross-core sum + divide.

Self-contained: hardcodes all shapes; no sibling imports.
"""
import math
import numpy as np

import concourse.bass as bass
import concourse.bacc as bacc
import concourse.tile as tile
from concourse import mybir, bass2jax
from concourse.masks import make_identity

AF = mybir.ActivationFunctionType
ALU = mybir.AluOpType
F32 = mybir.dt.float32
BF16 = mybir.dt.bfloat16
I32 = mybir.dt.int32

NC = 8          # cores
N = 4096        # tokens
T = N // NC     # tokens per core (512)
D = 256
H = 8
DH = 32
FF = 1024
L = 2
NAG = 512       # agents
WIN0, WIN_LEN = -20, 50
KAUG = WIN_LEN + 4 + 1   # 55 rows: onehot(50) + seq(4) + ones(1)
NKB = N // 128           # 32 key blocks
EPS = 1e-5


def _pe_table():
    t = np.arange(WIN0, WIN0 + WIN_LEN, dtype=np.float32)[:, None]
    div = np.exp(np.arange(0, D, 2, dtype=np.float32) * (-math.log(10000.0) / D))
    pe = np.zeros((WIN_LEN, D), np.float32)
    pe[:, 0::2] = np.sin(t * div)
    pe[:, 1::2] = np.cos(t * div)
    return pe


def build_nc(repeat=1):
    nc = bacc.Bacc("TRN2", target_bir_lowering=False, debug=False, num_devices=NC)

    # ---- DRAM I/O (identical across cores unless noted)
    seqT = nc.dram_tensor("seqT", [5, N], F32, kind="ExternalInput")       # scene.T, vel.T, ones
    ts = nc.dram_tensor("ts", [1, N], I32, kind="ExternalInput")
    seqTl = nc.dram_tensor("seqTl", [5, T], F32, kind="ExternalInput")     # per-core
    tsl = nc.dram_tensor("tsl", [1, T], I32, kind="ExternalInput")         # per-core
    agl = nc.dram_tensor("agl", [T], I32, kind="ExternalInput")            # per-core
    waug = nc.dram_tensor("waug", [KAUG, D], F32, kind="ExternalInput")    # [pe; fc_w; fc_b]
    wqkv = nc.dram_tensor("wqkv", [L, D, 3 * D], F32, kind="ExternalInput")
    bqkv = nc.dram_tensor("bqkv", [L, 3 * D], F32, kind="ExternalInput")
    wout = nc.dram_tensor("wout", [L, D, D], F32, kind="ExternalInput")
    bout = nc.dram_tensor("bout", [L, D], F32, kind="ExternalInput")
    w1 = nc.dram_tensor("w1", [L, D, FF], F32, kind="ExternalInput")
    b1 = nc.dram_tensor("b1", [L, FF], F32, kind="ExternalInput")
    w2 = nc.dram_tensor("w2", [L, FF, D], F32, kind="ExternalInput")
    b2 = nc.dram_tensor("b2", [L, D], F32, kind="ExternalInput")
    ln1g = nc.dram_tensor("ln1g", [L, D], F32, kind="ExternalInput")
    ln1b = nc.dram_tensor("ln1b", [L, D], F32, kind="ExternalInput")
    ln2g = nc.dram_tensor("ln2g", [L, D], F32, kind="ExternalInput")
    ln2b = nc.dram_tensor("ln2b", [L, D], F32, kind="ExternalInput")
    part = nc.dram_tensor("part", [NAG, D + 1], F32, kind="ExternalOutput")

    with tile.TileContext(nc) as tc:
        with (
            tc.tile_pool(name="const", bufs=1) as cp,
            tc.tile_pool(name="big", bufs=1) as bigp,
            tc.tile_pool(name="wp", bufs=2) as wp,
            tc.tile_pool(name="act", bufs=1) as ap,
            tc.tile_pool(name="pt", bufs=2) as ptp,
            tc.tile_pool(name="rows", bufs=4) as rowsp,
            tc.tile_pool(name="pp", bufs=2, space="PSUM") as pp,
            tc.tile_pool(name="scq", bufs=1, space="PSUM") as scqp,
            tc.tile_pool(name="av", bufs=2, space="PSUM") as avp_pool,
            tc.tile_pool(name="dram", bufs=1, space="DRAM") as dp,
        ):
            # ---- constants
            ident = cp.tile([128, 128], F32, name="ident")
            make_identity(nc, ident)
            ones512 = cp.tile([1, 512], F32, name="ones512")
            nc.vector.memset(ones512[:], 1.0)
            ones128r = cp.tile([1, 128], F32, name="ones128r")
            nc.vector.memset(ones128r[:], 1.0)
            ones32r = cp.tile([1, 32], F32, name="ones32r")
            nc.vector.memset(ones32r[:], 1.0)
            ones128c = cp.tile([128, 1], F32, name="ones128c")
            nc.vector.memset(ones128c[:], 1.0)
            eps1 = cp.tile([1, 1], F32, name="eps1")
            nc.vector.memset(eps1[:], EPS)
            eps128 = cp.tile([128, 1], F32, name="eps128")
            nc.vector.memset(eps128[:], EPS)
            iotwf = cp.tile([WIN_LEN, 1], F32, name="iotwf")
            nc.gpsimd.iota(iotwf[:], pattern=[[0, 1]], base=WIN0, channel_multiplier=1,
                           allow_small_or_imprecise_dtypes=True)
            iotagf = cp.tile([128, NAG], F32, name="iotagf")
            nc.gpsimd.iota(iotagf[:], pattern=[[1, NAG]], base=0, channel_multiplier=0,
                           allow_small_or_imprecise_dtypes=True)
            agf = cp.tile([128, 4], F32, name="agf")
            agi = cp.tile([128, 4], I32, name="agi")
            nc.sync.dma_start(agi[:], agl[:].rearrange("(c p) -> p c", p=128))
            nc.vector.tensor_copy(agf[:], agi[:])
            waug_sb = cp.tile([KAUG, D], F32, name="waug_sb")
            nc.sync.dma_start(waug_sb[:], waug[:])
            # final-LN scale/bias broadcast (token-major, layer L-1)
            ln2g_bc = cp.tile([128, D], F32, name="ln2g_bc")
            nc.sync.dma_start(ln2g_bc[:], ln2g[L - 1:L, :].to_broadcast((128, D)))
            ln2b_bc = cp.tile([128, D], F32, name="ln2b_bc")
            nc.sync.dma_start(ln2b_bc[:], ln2b[L - 1:L, :].to_broadcast((128, D)))

            # ---- persistent big tiles
            kt = [bigp.tile([128, N], BF16, name=f"kt{i}") for i in range(2)]
            vaug = bigp.tile([128, NKB, 33 * H], BF16, name="vaug")
            vaug_h = vaug[:].rearrange("p k (h c) -> p k h c", h=H)
            nc.vector.memset(vaug_h[:, :, :, 32:33], 1.0)

            for rep in range(repeat):
                # xT = local residual stream (dim-major, [2][128, T])
                xT = [ap.tile([128, T], F32, name=f"xT{i}_r{rep}", tag=f"xT{i}")
                      for i in range(2)]

                for l in range(L):
                    # ---------- phase 1: x_full + K/V/q builds ----------
                    with tc.tile_pool(name="xf", bufs=1) as xfp:
                        xF = [xfp.tile([128, N], F32, name=f"xF{i}_{l}_r{rep}", tag=f"xF{i}")
                              for i in range(2)]
                        if l == 0:
                            # build A (chunked) and x0 full + local
                            with tc.tile_pool(name="abuild", bufs=1) as abp:
                                chunks = [(ts, seqT, 2048 * i, 2048, xF, True) for i in range(2)]
                                chunks.append((tsl, seqTl, 0, T, xT, False))
                                for tsrc, ssrc, off, n_, dsts, full in chunks:
                                    tsb = abp.tile([WIN_LEN, n_], I32, name=f"tsb{off}_{full}_r{rep}", tag=f"tsb{full}")
                                    nc.sync.dma_start(tsb[:], tsrc[:, off:off + n_].to_broadcast((WIN_LEN, n_)))
                                    A = abp.tile([KAUG, n_], F32, name=f"A{off}_{full}_r{rep}", tag=f"A{full}")
                                    nc.vector.tensor_copy(A[:WIN_LEN, :], tsb[:])
                                    nc.vector.tensor_scalar(A[:WIN_LEN, :], A[:WIN_LEN, :], iotwf[:], None, op0=ALU.is_equal)
                                    nc.sync.dma_start(A[WIN_LEN:, :], ssrc[:, off:off + n_])
                                    for kd in range(2):
                                        for tb in range(n_ // 512):
                                            ps = pp.tile([128, 512], F32, name=f"x0ps_{off}_{full}{kd}_{tb}_r{rep}", tag="pp")
                                            nc.tensor.matmul(ps[:], waug_sb[:, 128 * kd:128 * (kd + 1)],
                                                             A[:, 512 * tb:512 * (tb + 1)], start=True, stop=True)
                                            dst = dsts[kd]
                                            nc.vector.tensor_copy(dst[:, off + 512 * tb:off + 512 * (tb + 1)] if full else dst[:, 512 * tb:512 * (tb + 1)], ps[:])
                        else:
                            # AllGather previous xT -> xF
                            agin = dp.tile([D, T], F32, name=f"agin_r{rep}", tag="agin")
                            agout = dp.tile([NC * D, T], F32, name=f"agout_r{rep}", tag="agout")
                            for kd in range(2):
                                nc.sync.dma_start(agin[128 * kd:128 * (kd + 1), :], xT[kd][:])
                            nc.gpsimd.collective_compute(
                                "AllGather", ALU.bypass,
                                replica_groups=[list(range(NC))],
                                ins=[agin[:].opt()], outs=[agout[:].opt()],
                            )
                            for r in range(NC):
                                for kd in range(2):
                                    nc.sync.dma_start(
                                        xF[kd][:, T * r:T * (r + 1)],
                                        agout[D * r + 128 * kd:D * r + 128 * (kd + 1), :])

                        # layer weights (bufs=2 tags -> layer l+1 can prefetch)
                        wqkv_sb = [wp.tile([128, 3 * D], F32, name=f"wqkv{i}_{l}_r{rep}", tag=f"wqkv{i}") for i in range(2)]
                        wout_sb = [wp.tile([128, D], F32, name=f"wout{i}_{l}_r{rep}", tag=f"wout{i}") for i in range(2)]
                        w1_sb = [wp.tile([128, FF], F32, name=f"w1_{i}_{l}_r{rep}", tag=f"w1_{i}") for i in range(2)]
                        w2_sb = [wp.tile([128, D], F32, name=f"w2_{i}_{l}_r{rep}", tag=f"w2_{i}") for i in range(8)]
                        for i in range(2):
                            nc.sync.dma_start(wqkv_sb[i][:], wqkv[l, 128 * i:128 * (i + 1), :])
                            nc.sync.dma_start(wout_sb[i][:], wout[l, 128 * i:128 * (i + 1), :])
                            nc.sync.dma_start(w1_sb[i][:], w1[l, 128 * i:128 * (i + 1), :])
                        for i in range(8):
                            nc.sync.dma_start(w2_sb[i][:], w2[l, 128 * i:128 * (i + 1), :])
                        bq_sb = wp.tile([1, 3 * D], F32, name=f"bq_{l}_r{rep}", tag="bq", bufs=1)
                        nc.sync.dma_start(bq_sb[:], bqkv[l:l + 1, :])
                        bo_sb = wp.tile([1, D], F32, name=f"bo_{l}_r{rep}", tag="bo", bufs=1)
                        nc.sync.dma_start(bo_sb[:], bout[l:l + 1, :])
                        b1_sb = wp.tile([1, FF], F32, name=f"b1_{l}_r{rep}", tag="b1", bufs=1)
                        nc.sync.dma_start(b1_sb[:], b1[l:l + 1, :])
                        b2_sb = wp.tile([1, D], F32, name=f"b2_{l}_r{rep}", tag="b2", bufs=1)
                        nc.sync.dma_start(b2_sb[:], b2[l:l + 1, :])
                        ln_sb = {}
                        for nm, dr_ in (("1g", ln1g), ("1b", ln1b), ("2g", ln2g), ("2b", ln2b)):
                            t_ = wp.tile([128, 2], F32, name=f"ln{nm}_{l}_r{rep}", tag=f"ln{nm}", bufs=1)
                            nc.sync.dma_start(t_[:], dr_[l, :].rearrange("(t p) -> p t", p=128))
                            ln_sb[nm] = t_

                        # K^T full: [256 kdim, 4096 keys]
                        for kd in range(2):
                            for tb in range(NKB // 4):
                                ps = pp.tile([128, 512], F32, name=f"ktps{kd}_{tb}_{l}_r{rep}", tag="pp")
                                for i in range(2):
                                    nc.tensor.matmul(ps[:], wqkv_sb[i][:, D + 128 * kd:D + 128 * (kd + 1)],
                                                     xF[i][:, 512 * tb:512 * (tb + 1)],
                                                     start=(i == 0), stop=False)
                                nc.tensor.matmul(ps[:], bq_sb[:, D + 128 * kd:D + 128 * (kd + 1)], ones512[:],
                                                 start=False, stop=True)
                                nc.vector.tensor_copy(kt[kd][:, 512 * tb:512 * (tb + 1)], ps[:])
                        # V (token-major) + ones col, interleaved per head
                        for kb in range(NKB):
                            ps = pp.tile([128, 256], F32, name=f"vps{kb}_{l}_r{rep}", tag="pp")
                            for i in range(2):
                                nc.tensor.matmul(ps[:], xF[i][:, 128 * kb:128 * (kb + 1)],
                                                 wqkv_sb[i][:, 2 * D:3 * D], start=(i == 0), stop=False)
                            nc.tensor.matmul(ps[:], ones128r[:], bq_sb[:, 2 * D:3 * D], start=False, stop=True)
                            nc.vector.tensor_copy(
                                vaug_h[:, kb, :, :32],
                                ps[:].rearrange("p (h c) -> p h c", c=32))
                        # q^T local
                        qt = [ap.tile([128, T], BF16, name=f"qt{i}_{l}_r{rep}", tag=f"qt{i}") for i in range(2)]
                        for kd in range(2):
                            ps = pp.tile([128, 512], F32, name=f"qps{kd}_{l}_r{rep}", tag="pp")
                            for i in range(2):
                                nc.tensor.matmul(ps[:], wqkv_sb[i][:, 128 * kd:128 * (kd + 1)], xT[i][:],
                                                 start=(i == 0), stop=False)
                            nc.tensor.matmul(ps[:], bq_sb[:, 128 * kd:128 * (kd + 1)], ones512[:],
                                             start=False, stop=True)
                            nc.vector.tensor_copy(qt[kd][:], ps[:])

                    # ---------- phase 2: attention ----------
                    ot = [ap.tile([128, T], F32, name=f"ot{i}_{l}_r{rep}", tag=f"ot{i}") for i in range(2)]
                    for h in range(H):
                        hq, hr = divmod(h, 4)
                        avps = avp_pool.tile([33, 512], F32, name=f"avps{h}_{l}_r{rep}", tag="av")
                        for qd in range(8):
                            scq = scqp.tile([128, 4, 512], F32, name=f"scq{h}_{qd}_{l}_r{rep}", tag="scq")
                            for j in range(4):
                                kb = qd * 4 + j
                                nc.tensor.matmul(
                                    scq[:, j, :],
                                    kt[hq][32 * hr:32 * (hr + 1), 128 * kb:128 * (kb + 1)],
                                    qt[hq][32 * hr:32 * (hr + 1), :],
                                    start=True, stop=True,
                                    tile_position=(32 * hr, 0))
                            pt = ptp.tile([128, 4, 512], BF16, name=f"pt{h}_{qd}_{l}_r{rep}", tag="pt")
                            nc.scalar.activation(pt[:, :, :], scq[:, :, :], AF.Exp,
                                                 scale=1.0 / math.sqrt(DH))
                            for j in range(4):
                                kb = qd * 4 + j
                                nc.tensor.matmul(avps[:], vaug_h[:, kb, h, :], pt[:, j, :],
                                                 start=(qd == 0 and j == 0),
                                                 stop=(qd == 7 and j == 3))
                        # eviction: normalize rows 0..31 by row 32
                        dr = rowsp.tile([1, 512], F32, name=f"dr{h}_{l}_r{rep}", tag="rows")
                        nc.vector.reciprocal(dr[:], avps[32:33, :])
                        dbc = pp.tile([32, 512], F32, name=f"dbc{h}_{l}_r{rep}", tag="pp")
                        nc.tensor.matmul(dbc[:], ones32r[:], dr[:], start=True, stop=True)
                        oslice = ot[hq][32 * hr:32 * (hr + 1), :]
                        nc.vector.tensor_copy(oslice, avps[:32, :])
                        nc.vector.tensor_mul(oslice, oslice, dbc[:])

                    # ---------- phase 3: out proj + LN1 ----------
                    r1 = [ap.tile([128, T], F32, name=f"r1_{i}_{l}_r{rep}", tag=f"r{i}") for i in range(2)]
                    for kd in range(2):
                        ps = pp.tile([128, 512], F32, name=f"ops{kd}_{l}_r{rep}", tag="pp")
                        for i in range(2):
                            nc.tensor.matmul(ps[:], wout_sb[i][:, 128 * kd:128 * (kd + 1)], ot[i][:],
                                             start=(i == 0), stop=False)
                        nc.tensor.matmul(ps[:], bo_sb[:, 128 * kd:128 * (kd + 1)], ones512[:],
                                         start=False, stop=True)
                        nc.vector.tensor_add(r1[kd][:], xT[kd][:], ps[:])
                    xT = [ap.tile([128, T], F32, name=f"xTa{i}_{l}_r{rep}", tag=f"xT{i}")
                          for i in range(2)]
                    _dim_major_ln(nc, tc, pp, avp_pool, ap, rowsp, r1, xT, ln_sb["1g"], ln_sb["1b"],
                                  ones128c, ones128r, eps1, f"ln1_{l}_r{rep}")

                    # ---------- phase 4: FF + LN2 ----------
                    f2ps = [avp_pool.tile([128, 512], F32, name=f"f2ps{kd}_{l}_r{rep}", tag="av") for kd in range(2)]
                    ht = [ap.tile([128, T], F32, name=f"ht{i}_{l}_r{rep}", tag=f"ht{i}") for i in range(4)]
                    for half in range(2):
                        for fi in range(4):
                            ft = half * 4 + fi
                            ps = pp.tile([128, 512], F32, name=f"f1ps{ft}_{l}_r{rep}", tag="pp")
                            for i in range(2):
                                nc.tensor.matmul(ps[:], w1_sb[i][:, 128 * ft:128 * (ft + 1)], xT[i][:],
                                                 start=(i == 0), stop=False)
                            nc.tensor.matmul(ps[:], b1_sb[:, 128 * ft:128 * (ft + 1)], ones512[:],
                                             start=False, stop=True)
                            nc.vector.tensor_scalar(ht[fi][:], ps[:], 0.0, None, op0=ALU.max)
                        for kd in range(2):
                            for fi in range(4):
                                ft = half * 4 + fi
                                nc.tensor.matmul(f2ps[kd][:], w2_sb[ft][:, 128 * kd:128 * (kd + 1)], ht[fi][:],
                                                 start=(half == 0 and fi == 0), stop=False)
                    r2 = [ap.tile([128, T], F32, name=f"r2_{i}_{l}_r{rep}", tag=f"r{i}") for i in range(2)]
                    for kd in range(2):
                        nc.tensor.matmul(f2ps[kd][:], b2_sb[:, 128 * kd:128 * (kd + 1)], ones512[:],
                                         start=False, stop=True)
                        nc.vector.tensor_add(r2[kd][:], xT[kd][:], f2ps[kd][:])

                    if l < L - 1:
                        xT = [ap.tile([128, T], F32, name=f"xTb{i}_{l}_r{rep}", tag=f"xT{i}")
                              for i in range(2)]
                        _dim_major_ln(nc, tc, pp, avp_pool, ap, rowsp, r2, xT, ln_sb["2g"], ln_sb["2b"],
                                      ones128c, ones128r, eps1, f"ln2_{l}_r{rep}")

                # ---------- final: transpose + token-major LN2 + segment sums ----------
                fin_cm = tc.tile_pool(name="fin", bufs=1)
                finp = fin_cm.__enter__()
                xa = [finp.tile([128, D + 1], F32, name=f"xa{c}_r{rep}", tag=f"xa{c}") for c in range(4)]
                oh = [finp.tile([128, NAG], F32, name=f"oh{c}_r{rep}", tag=f"oh{c}") for c in range(4)]
                for c in range(4):
                    xtok = finp.tile([128, D], F32, name=f"xtok{c}_r{rep}", tag="xtok")
                    for kd in range(2):
                        tpp = pp.tile([128, 128], F32, name=f"tp{c}_{kd}_r{rep}", tag="pp")
                        nc.tensor.transpose(tpp[:], r2[kd][:, 128 * c:128 * (c + 1)], ident[:])
                        nc.vector.tensor_copy(xtok[:, 128 * kd:128 * (kd + 1)], tpp[:])
                    stats = finp.tile([128, 6], F32, name=f"st{c}_r{rep}", tag="st")
                    nc.vector.bn_stats(stats[:], xtok[:])
                    mv = finp.tile([128, 2], F32, name=f"mv{c}_r{rep}", tag="mv")
                    nc.vector.bn_aggr(mv[:], stats[:])
                    sd = finp.tile([128, 1], F32, name=f"sd{c}_r{rep}", tag="sd")
                    nc.scalar.activation(sd[:], mv[:, 1:2], AF.Sqrt, bias=eps128[:])
                    rs = finp.tile([128, 1], F32, name=f"rs{c}_r{rep}", tag="rs")
                    nc.vector.reciprocal(rs[:], sd[:])
                    nc.vector.tensor_scalar(xtok[:], xtok[:], mv[:, 0:1], rs[:],
                                            op0=ALU.subtract, op1=ALU.mult)
                    nc.vector.tensor_mul(xtok[:], xtok[:], ln2g_bc[:])
                    nc.vector.tensor_add(xa[c][:, :D], xtok[:], ln2b_bc[:])
                    nc.vector.memset(xa[c][:, D:D + 1], 1.0)
                    nc.vector.tensor_scalar(oh[c][:], iotagf[:], agf[:, c:c + 1], None, op0=ALU.is_equal)
                for as_ in range(4):
                    ps = pp.tile([128, D + 1], F32, name=f"segps{as_}_r{rep}", tag="pp")
                    for c in range(4):
                        nc.tensor.matmul(ps[:], oh[c][:, 128 * as_:128 * (as_ + 1)], xa[c][:],
                                         start=(c == 0), stop=(c == 3))
                    seg_sb = finp.tile([128, D + 1], F32, name=f"seg{as_}_r{rep}", tag="segsb", bufs=2)
                    nc.vector.tensor_copy(seg_sb[:], ps[:])
                    nc.sync.dma_start(part[128 * as_:128 * (as_ + 1), :], seg_sb[:])
                fin_cm.__exit__(None, None, None)

    nc.compile()
    return nc


def _dim_major_ln(nc, tc, pp, av_pool, ap, rowsp, rin, xout, g_sb, b_sb, ones128c, ones128r, eps1, sfx):
    """LayerNorm over the partition (feature) axis for dim-major [2][128, T] tiles."""
    sq = [ap.tile([128, T], F32, name=f"sq{i}_{sfx}", tag=f"lntmp{i}") for i in range(2)]
    for i in range(2):
        nc.vector.tensor_mul(sq[i][:], rin[i][:], rin[i][:])
    s1 = pp.tile([1, T], F32, name=f"s1_{sfx}", tag="pp")
    s2 = pp.tile([1, T], F32, name=f"s2_{sfx}", tag="pp")
    for i in range(2):
        nc.tensor.matmul(s1[:], ones128c[:], rin[i][:], start=(i == 0), stop=(i == 1))
        nc.tensor.matmul(s2[:], ones128c[:], sq[i][:], start=(i == 0), stop=(i == 1))
    A = rowsp.tile([1, T], F32, name=f"lnA_{sfx}", tag="rows")
    B = rowsp.tile([1, T], F32, name=f"lnB_{sfx}", tag="rows")
    C = rowsp.tile([1, T], F32, name=f"lnC_{sfx}", tag="rows")
    nc.vector.tensor_scalar(A[:], s1[:], 1.0 / D, None, op0=ALU.mult)      # mean
    nc.vector.tensor_scalar(B[:], s2[:], 1.0 / D, None, op0=ALU.mult)      # E[x^2]
    nc.vector.tensor_mul(C[:], A[:], A[:])                                  # m^2
    nc.vector.tensor_sub(B[:], B[:], C[:])                                  # var
    nc.scalar.activation(C[:], B[:], AF.Sqrt, bias=eps1[:])                 # sd
    nc.vector.reciprocal(B[:], C[:])                                        # rstd
    nc.vector.tensor_mul(A[:], A[:], B[:])
    nc.vector.tensor_scalar(A[:], A[:], -1.0, None, op0=ALU.mult)           # -m*rstd
    ba = pp.tile([128, T], F32, name=f"ba_{sfx}", tag="pp")
    nc.tensor.matmul(ba[:], ones128r[:], B[:], start=True, stop=True)
    bc = av_pool.tile([128, T], F32, name=f"bc_{sfx}", tag="av")
    nc.tensor.matmul(bc[:], ones128r[:], A[:], start=True, stop=True)
    for i in range(2):
        t1 = ap.tile([128, T], F32, name=f"t1_{i}_{sfx}", tag=f"lntmp{i}")
        nc.vector.tensor_mul(t1[:], rin[i][:], ba[:])
        nc.vector.tensor_add(t1[:], t1[:], bc[:])
        nc.vector.tensor_scalar(xout[i][:], t1[:], g_sb[:, i:i + 1], b_sb[:, i:i + 1],
                                op0=ALU.mult, op1=ALU.add)


# ---------------- host side ----------------

_CACHE = {}


def _get_exec(repeat):
    """Compile the bass program (once per repeat count) and build a jit-once
    PJRT executor. Returns dict with fn/in_names/out_names/out_avals."""
    if repeat in _CACHE:
        return _CACHE[repeat]
    import jax
    from jax.sharding import Mesh, PartitionSpec
    from jax.experimental.shard_map import shard_map

    nc = build_nc(repeat)
    bass2jax.install_neuronx_cc_hook()
    partition_name = nc.partition_id_tensor.name if nc.partition_id_tensor else None
    in_names, out_names, out_avals = [], [], []
    for alloc in nc.m.functions[0].allocations:
        if not isinstance(alloc, mybir.MemoryLocationSet):
            continue
        name = alloc.memorylocations[0].name
        if alloc.kind == "ExternalInput":
            if name != partition_name:
                in_names.append(name)
        elif alloc.kind == "ExternalOutput":
            out_names.append(name)
            shape = tuple(alloc.tensor_shape)
            dtype = mybir.dt.np(alloc.dtype)
            out_avals.append(jax.core.ShapedArray(shape, dtype))
    in_names_all = list(in_names) + list(out_names)
    if partition_name is not None:
        in_names_all.append(partition_name)

    def _body(*args):
        operands = list(args)
        if partition_name is not None:
            operands.append(bass2jax.partition_id_tensor())
        outs = bass2jax._bass_exec_p.bind(
            *operands,
            out_avals=tuple(out_avals),
            in_names=tuple(in_names_all),
            out_names=tuple(out_names),
            lowering_input_output_aliases=(),
            sim_require_finite=False,
            sim_require_nnan=False,
            nc=nc,
        )
        return tuple(outs)

    devices = jax.devices()[:NC]
    mesh = Mesh(np.asarray(devices), ("core",))
    fn = jax.jit(
        shard_map(_body, mesh=mesh,
                  in_specs=(PartitionSpec("core"),) * (len(in_names) + len(out_names)),
                  out_specs=(PartitionSpec("core"),) * len(out_names),
                  check_rep=False),
        keep_unused=True,
    )
    ex = dict(fn=fn, in_names=in_names, out_names=out_names, out_avals=out_avals)
    _CACHE[repeat] = ex
    return ex


def _device_args(ex, in_maps):
    import jax
    per_core = [[np.asarray(m[name]) for name in ex["in_names"]] for m in in_maps]
    concat_in = [np.concatenate([per_core[c][i] for c in range(NC)], axis=0)
                 for i in range(len(ex["in_names"]))]
    concat_zeros = [np.zeros((NC * a.shape[0], *a.shape[1:]), a.dtype)
                    for a in ex["out_avals"]]
    return [jax.device_put(a) for a in (concat_in + concat_zeros)]


def _run_exec(ex, args_dev):
    outs = [np.asarray(o) for o in ex["fn"](*args_dev)]
    return [
        {name: outs[i].reshape(NC, *ex["out_avals"][i].shape)[c]
         for i, name in enumerate(ex["out_names"])}
        for c in range(NC)
    ]


def _prep_in_maps(inputs):
    gi = {k: np.asarray(v) for k, v in inputs.items()}
    scene = gi["pre_sequence_scene_norm"].astype(np.float32)
    vel = gi["pre_vel_seq"].astype(np.float32)
    ts = gi["pre_timesteps"].astype(np.int32)
    ag = gi["pre_agents"].astype(np.int32)
    seqT = np.concatenate([scene.T, vel.T, np.ones((1, N), np.float32)], axis=0)  # [5, N]
    waug = np.concatenate([_pe_table(),
                           gi["input_fc_w"].astype(np.float32),
                           gi["input_fc_b"].astype(np.float32)[None, :]], axis=0)  # [55, 256]
    common = dict(
        seqT=np.ascontiguousarray(seqT),
        ts=ts.reshape(1, N),
        waug=np.ascontiguousarray(waug),
        wqkv=np.ascontiguousarray(gi["qkv_w"].astype(np.float32)),
        bqkv=np.ascontiguousarray(gi["qkv_b"].astype(np.float32)),
        wout=np.ascontiguousarray(gi["out_w"].astype(np.float32)),
        bout=np.ascontiguousarray(gi["out_b"].astype(np.float32)),
        w1=np.ascontiguousarray(gi["ff1_w"].astype(np.float32)),
        b1=np.ascontiguousarray(gi["ff1_b"].astype(np.float32)),
        w2=np.ascontiguousarray(gi["ff2_w"].astype(np.float32)),
        b2=np.ascontiguousarray(gi["ff2_b"].astype(np.float32)),
        ln1g=np.ascontiguousarray(gi["ln1_g"].astype(np.float32)),
        ln1b=np.ascontiguousarray(gi["ln1_b"].astype(np.float32)),
        ln2g=np.ascontiguousarray(gi["ln2_g"].astype(np.float32)),
        ln2b=np.ascontiguousarray(gi["ln2_b"].astype(np.float32)),
    )
    in_maps = []
    for c in range(NC):
        sl = slice(T * c, T * (c + 1))
        m = dict(common)
        m["seqTl"] = np.ascontiguousarray(seqT[:, sl])
        m["tsl"] = np.ascontiguousarray(ts[sl].reshape(1, T))
        m["agl"] = np.ascontiguousarray(ag[sl])
        in_maps.append(m)
    return in_maps


def _combine(results):
    acc = np.zeros((NAG, D + 1), np.float64)
    for c in range(NC):
        acc += results[c]["part"].astype(np.float64)
    sums = acc[:, :D]
    cnt = np.maximum(acc[:, D:D + 1], 1.0)
    return (sums / cnt).astype(np.float32)


def kernel(**inputs):
    ex = _get_exec(1)
    args_dev = _device_args(ex, _prep_in_maps(inputs))
    return _combine(_run_exec(ex, args_dev))


if __name__ == "__main__":
    import reference
    inputs = {k: np.asarray(v) for k, v in reference.setup_inputs().items()}
    out = kernel(**inputs)
    exp = np.asarray(reference.reference(**inputs))
    err = np.abs(out - exp).max() / (np.abs(exp).max() + 1e-12)
    print("max abs err:", np.abs(out - exp).max(), "rel:", err)
